# revision 1
# baseline (speedup 1.0000x reference)
"""Distributed multi-head attention for TRN2 (8 NeuronCores).

Reference computation (per batch b):
    qkv = x @ w_qkv.T                         # (N, 3C)
    q, k, v = split/reshape to (H, N, D)
    attn = softmax(q @ k.T * D**-0.5)         # per head
    out = (attn @ v) reassembled to (N, C)
    out = out @ w_proj.T + b_proj

Sharding: 8 cores = 4 batches x 2 query-halves. Each core computes k/v
for all 2048 tokens of its batch (duplicated across the 2 cores of a
batch - cheaper than communicating), q for its own 1024 tokens, the
full attention for all 12 heads over its 1024 queries, and the output
projection. No collectives.

Layout strategy (all chosen so no on-chip transposes are needed):
  - host passes x^T and w_qkv^T so projections contract over partitions
  - q,k are produced "d-major" ([head-dim, tokens]) via out^T-form
    matmuls; scores are computed transposed ([keys, queries]) which is
    exactly the layout attn@v consumes as its stationary-side operand
  - softmax needs no max-subtraction (scores ~ N(0,1), fp32 exp range)
  - the denominator rides along as a ones-column appended to v (M=65
    matmuls); normalization uses a K=1 ones-matmul to broadcast 1/denom
    across partitions
  - all matmuls in bf16 (PSUM accumulation is fp32); softmax exp runs
    on the scalar (ACT) engine from PSUM f32, writing bf16 probs

Schedule: the ACT engine (softmax exp, ~1us per 128x1024 tile) is the
steady-state bottleneck; everything else hides under it. Attention runs
as 12 passes (head pair x query half). Per pass and k-block: the two
heads' score matmuls write one shared PSUM tile, alternating PE row
groups (base partition 0/64) so they run concurrently; exp(kb) overlaps
scores(kb+1) via two PSUM slots; attn@v lags by one k-block. The query
halving keeps the pass's PSUM footprint at 6 banks, leaving 2 banks for
"filler" projection work that keeps the PE busy (and its HAM clock
warm): pass 0 produces v block kb just-in-time in step kb, passes 1-5
drain the k/q blocks of later pairs. The per-pass normalization
epilogue is split so its PE part lands inside the next pass.

Self-contained: hardcodes B=4, N=2048, C=768, H=12, D=64.
"""

import numpy as np
import ml_dtypes

import concourse.bass as bass
import concourse.mybir as mybir
from concourse import bacc
from concourse.tile import TileContext
from concourse.bass_utils import run_bass_kernel_spmd

F32 = mybir.dt.float32
BF16 = mybir.dt.bfloat16
EXP = mybir.ActivationFunctionType.Exp

B, N, C = 4, 2048, 768
H, D = 12, 64
SCALE = float(D) ** -0.5  # 0.125
NQ = N // 2  # queries per core: 1024
CB = C // 128  # 6 c-chunks
TB = N // 128  # 16 token blocks
HB = H // 2  # 6 head pairs
VW = H * (D + 1)  # 780: v block width with ones columns

N_CORES = 8

# w_qkv columns, grouped in the order the projection units consume them:
# pair-0 k/q, all v, then k/q for pairs 1..5. Each group holds its column
# range for all six 128-row input chunks, contiguously.
_WQ_GROUPS = [(C, 128), (0, 128), (2 * C, C)]
for _ob in range(1, CB):
    _WQ_GROUPS.append((C + _ob * 128, 128))
    _WQ_GROUPS.append((_ob * 128, 128))
_WQ_BASE = {}
_cur = 0
for _o0, _w in _WQ_GROUPS:
    _WQ_BASE[_o0] = (_cur, _w)
    _cur += CB * _w


def _build():
    nc = bacc.Bacc(None, target_bir_lowering=False)

    # host-packed SBUF images: xTp cols = [tch][ci][t]; wqp cols grouped
    # in consumption order (see _WQ_GROUPS)
    xTp = nc.declare_dram_parameter("xTp", [128, CB * N], BF16, isOutput=False)
    wqp = nc.declare_dram_parameter("wqp", [128, CB * 3 * C], BF16, isOutput=False)
    wprojp = nc.declare_dram_parameter("wprojp", [128, CB * C], BF16, isOutput=False)
    biasp = nc.declare_dram_parameter("biasp", [128, CB], F32, isOutput=False)
    outT = nc.declare_dram_parameter("outT", [C, NQ], BF16, isOutput=True)

    with TileContext(nc) as tc:
        with (
            tc.tile_pool(name="per", bufs=1) as per,
            tc.tile_pool(name="p23", bufs=1) as p23,
            tc.tile_pool(name="hp", bufs=8) as hp,
            tc.tile_pool(name="mi", bufs=3) as mi,
            tc.tile_pool(name="op", bufs=2) as op_pool,
            tc.tile_pool(name="ps", bufs=2, space="PSUM") as ps2,
        ):
            # ---- persistent tiles -------------------------------------
            qT_sb = per.tile([128, CB * NQ], BF16)  # q^T  [2 heads/blk, 1024]
            kT_sb = per.tile([128, CB * N], BF16)  # k^T  [2 heads/blk, 2048]
            vaug_sb = per.tile([128, TB * VW], BF16)  # v + ones cols
            bias_sb = per.tile([128, CB], F32)
            ones_sb = per.tile([1, 64], BF16)
            attnT_sb = p23.tile([128, CB * NQ], BF16)  # attn out^T
            wproj_sb = p23.tile([128, CB * C], BF16)

            nc.vector.memset(ones_sb[:, :], 1.0)
            # ones columns of vaug: col 64 of each 65-wide head slot
            vaug_ones = vaug_sb[:, :].rearrange(
                "p (t h x) -> p t h x", t=TB, h=H, x=D + 1
            )[:, :, :, D : D + 1]
            nc.vector.memset(vaug_ones, 1.0)

            # weights + activations pools, closed once the projection
            # filler has consumed them
            wqxt = (tc.tile_pool(name="wq", bufs=1), tc.tile_pool(name="xt", bufs=4))
            wq_pool = wqxt[0].__enter__()
            xt_pool = wqxt[1].__enter__()

            wqkv_sb = wq_pool.tile([128, CB * 3 * C], BF16)
            xts = [
                xt_pool.tile([128, CB * 512], BF16, tag="xt", name=f"xt{t}")
                for t in range(4)
            ]

            def _dma_xt(tch):
                nc.sync.dma_start(
                    out=xts[tch][:, :],
                    in_=xTp[:, tch * CB * 512 : (tch + 1) * CB * 512],
                )

            def _dma_wq(gi):
                o0, w = _WQ_GROUPS[gi]
                base, _ = _WQ_BASE[o0]
                nc.sync.dma_start(
                    out=wqkv_sb[:, base : base + CB * w],
                    in_=wqp[:, base : base + CB * w],
                )

            # consumption order: chunk 0 + pair-0 k/q cols first, then the
            # remaining token chunks, v cols, later pairs' k/q cols
            _dma_xt(0)
            _dma_wq(0)
            _dma_wq(1)
            for t in range(1, 4):
                _dma_xt(t)
            for gi in range(2, len(_WQ_GROUPS)):
                _dma_wq(gi)

            def wq(ci, o0, width):
                if o0 >= 2 * C:
                    base, gw = _WQ_BASE[2 * C]
                    off = o0 - 2 * C
                else:
                    base, gw = _WQ_BASE[o0]
                    off = 0
                return wqkv_sb[:, base + ci * gw + off : base + ci * gw + off + width]

            # phase-2/3-only weights: after the critical-path DMAs
            nc.sync.dma_start(out=bias_sb[:, :], in_=biasp[:, :])
            nc.sync.dma_start(out=wproj_sb[:, :], in_=wprojp[:, :])

            # ---- projection work units (PE filler) --------------------
            def kq_unit(ob, tch, is_q):
                """one k^T (or q^T) block: out-dims block ob, 512 tokens"""
                t0 = tch * 512
                kind = "q" if is_q else "k"
                psv = ps2.tile(
                    [128, 512], F32, tag="psV", bufs=2, name=f"{kind}{ob}_{tch}"
                )
                for ci in range(CB):
                    nc.tensor.matmul(
                        psv[:, :],
                        wq(ci, (0 if is_q else C) + ob * 128, 128),
                        xts[tch][:, ci * 512 : (ci + 1) * 512],
                        start=(ci == 0),
                        stop=(ci == CB - 1),
                    )
                if is_q:
                    nc.vector.tensor_copy(
                        qT_sb[:, ob * NQ + t0 : ob * NQ + t0 + 512], psv[:, :]
                    )
                else:
                    nc.vector.tensor_copy(
                        kT_sb[:, ob * N + t0 : ob * N + t0 + 512], psv[:, :]
                    )

            def v_unit(t128, o0, w):
                """one v unit: 128 tokens x [o0, o0+w) v-dims, written
                (bf16) into the vaug slot layout"""
                tch, tb = divmod(t128, 4)
                psv = ps2.tile(
                    [128, 512], F32, tag="psV", bufs=2, name=f"v{t128}_{o0}"
                )
                for ci in range(CB):
                    nc.tensor.matmul(
                        psv[:, :w],
                        xts[tch][:, ci * 512 + tb * 128 : ci * 512 + (tb + 1) * 128],
                        wq(ci, 2 * C + o0, w),
                        start=(ci == 0),
                        stop=(ci == CB - 1),
                    )
                nh = w // D
                src = psv[:, :w].rearrange("p (h x) -> p h x", x=D)
                h0 = o0 // D
                base = t128 * VW + h0 * (D + 1)
                dst = vaug_sb[:, base : base + nh * (D + 1)].rearrange(
                    "p (h x) -> p h x", x=D + 1
                )[:, :, :D]
                nc.vector.tensor_copy(dst, src)

            # remaining k/q blocks, drained by the pass fillers in order;
            # block hb is always complete before pair hb's first pass
            kq_queue = []
            for ob in range(1, CB):
                for tch in range(4):
                    kq_queue.append((ob, tch, False))
                for tch in range(2):
                    kq_queue.append((ob, tch, True))

            def fill_kq():
                if kq_queue:
                    ob_, tch_, is_q_ = kq_queue.pop(0)
                    kq_unit(ob_, tch_, is_q_)

            # ---- attention machinery ----------------------------------
            def epi_pe(hb_, qc_, outs_):
                """PE part of a pass's normalization epilogue. The two
                heads' 1/denom broadcasts go to different column strips of
                one PSUM tile (col tiling) so they run concurrently."""
                psb = ps2.tile(
                    [128, 512], F32, tag="psV", bufs=2,
                    name=f"psb{hb_}_{qc_}",
                )
                for hh_ in range(2):
                    nc.tensor.matmul(
                        psb[64 * hh_ : 64 * hh_ + 64, :],
                        ones_sb[:, :],
                        outs_[hh_][1][:, :],
                        start=True,
                        stop=True,
                    )
                for hh_ in range(2):
                    nc.vector.tensor_mul(
                        attnT_sb[
                            64 * hh_ : 64 * hh_ + 64,
                            hb_ * NQ + qc_ * 512 : hb_ * NQ + (qc_ + 1) * 512,
                        ],
                        psb[64 * hh_ : 64 * hh_ + 64, :],
                        outs_[hh_][0][:, :],
                    )

            def emit_pass(hb, qc, pend, filler=None):
                """One (head pair, query half) attention pass."""
                q0 = hb * NQ + qc * 512
                accs = [
                    ps2.tile(
                        [128, 512], F32, tag="psA", bufs=2,
                        name=f"acc{hb}_{qc}_{i}",
                    )
                    for i in range(2)
                ]
                def av_mms(pkb, ppb):
                    for hh in range(2):
                        vs = pkb * VW + (2 * hb + hh) * (D + 1)
                        nc.tensor.matmul(
                            accs[hh][0:65, :],
                            vaug_sb[:, vs : vs + D + 1],
                            ppb[:, hh * 512 : (hh + 1) * 512],
                            start=(pkb == 0),
                            stop=(pkb == TB - 1),
                        )

                # two k-blocks per step: the 4 score matmuls form an
                # alternating row-group run so their weight loads pipeline
                prev = []
                for kb2 in range(0, TB, 2):
                    scs = []
                    for kb in (kb2, kb2 + 1):
                        sc = ps2.tile(
                            [128, NQ], F32, tag="psS", bufs=2,
                            name=f"sc{hb}_{qc}_{kb}",
                        )
                        for hh in range(2):
                            p0 = 64 * hh
                            nc.tensor.matmul(
                                sc[:, hh * 512 : (hh + 1) * 512],
                                kT_sb[
                                    p0 : p0 + 64,
                                    hb * N + kb * 128 : hb * N + (kb + 1) * 128,
                                ],
                                qT_sb[p0 : p0 + 64, q0 : q0 + 512],
                                start=True,
                                stop=True,
                                tile_position=(p0, 0),
                            )
                        scs.append(sc)
                    if filler is not None:
                        filler(kb2)
                        filler(kb2 + 1)
                    for pkb, ppb in prev:
                        av_mms(pkb, ppb)
                    prev = []
                    for i, kb in enumerate((kb2, kb2 + 1)):
                        pb = hp.tile([128, NQ], BF16, tag="probs")
                        nc.scalar.activation(
                            pb[:, :], scs[i][:, :], EXP, scale=SCALE
                        )
                        prev.append((kb, pb))
                    if kb2 == 2 and pend is not None:
                        epi_pe(*pend)
                        pend = None
                # drain attn@v for the last two k-blocks
                for pkb, ppb in prev:
                    av_mms(pkb, ppb)
                # epilogue DVE part: drain accumulators + 1/denominator
                outs = []
                for hh in range(2):
                    acc = accs[hh]
                    cpy = mi.tile([64, 512], F32, tag="cpy")
                    nc.vector.tensor_copy(cpy[:, :], acc[0:64, :])
                    den = mi.tile([1, 512], F32, tag="den")
                    nc.vector.tensor_copy(den[:, :], acc[64:65, :])
                    rec = mi.tile([1, 512], F32, tag="rec")
                    nc.vector.reciprocal_approx_fast(rec[:, :], den[:, :])
                    row = mi.tile([1, 512], BF16, tag="row")
                    nc.vector.tensor_copy(row[:, :], rec[:, :])
                    outs.append((cpy, row))
                return (hb, qc, outs)

            # ---- pre-phase: k/q blocks for head pair 0 ----------------
            for tch in range(4):
                kq_unit(0, tch, False)
                if tch < 2:
                    kq_unit(0, tch, True)

            # ---- phase 2: 12 passes -----------------------------------
            # pass 0 produces v just-in-time (block kb in step kb, one
            # step before attn@v needs it); passes 1-5 drain kq_queue
            def fill_v(kb):
                v_unit(kb, 0, 512)
                v_unit(kb, 512, 256)
                if kb % 8 == 7:
                    fill_kq()

            def fill_k(kb):
                if kb % 3 == 0:
                    fill_kq()

            pend = emit_pass(0, 0, None, filler=fill_v)
            for pi in range(1, 2 * HB):
                hb, qc = divmod(pi, 2)
                filler = fill_k if pi <= 5 else None
                pend = emit_pass(hb, qc, pend, filler=filler)
            epi_pe(*pend)
            assert not kq_queue

            wqxt[1].__exit__(None, None, None)
            wqxt[0].__exit__(None, None, None)

            # ---- phase 3: output projection (out^T form) --------------
            for ob in range(CB):
                psp = ps2.tile([128, NQ], F32, tag="psS", name=f"prj{ob}")
                for cb in range(CB):
                    for qc in range(2):
                        nc.tensor.matmul(
                            psp[:, qc * 512 : (qc + 1) * 512],
                            wproj_sb[:, cb * C + ob * 128 : cb * C + (ob + 1) * 128],
                            attnT_sb[:, cb * NQ + qc * 512 : cb * NQ + (qc + 1) * 512],
                            start=(cb == 0),
                            stop=(cb == CB - 1),
                        )
                ot = op_pool.tile([128, NQ], BF16, tag="out")
                nc.vector.tensor_scalar_add(
                    ot[:, :], psp[:, :], bias_sb[:, ob : ob + 1]
                )
                nc.sync.dma_start(
                    out=outT[ob * 128 : (ob + 1) * 128, :], in_=ot[:, :]
                )

    nc.finalize()
    return nc


_NC_CACHE = []


def _get_nc():
    if not _NC_CACHE:
        _NC_CACHE.append(_build())
    return _NC_CACHE[0]


def kernel(x, w_qkv, w_proj, b_proj):
    x = np.asarray(x, dtype=np.float32)
    w_qkv = np.asarray(w_qkv, dtype=np.float32)
    w_proj = np.asarray(w_proj, dtype=np.float32)
    b_proj = np.asarray(b_proj, dtype=np.float32)

    nc = _get_nc()

    wqkvT = w_qkv.T.astype(ml_dtypes.bfloat16)  # [C, 3C]
    wq3 = np.ascontiguousarray(wqkvT).reshape(CB, 128, 3 * C)  # [ci, p, o]
    wqp = np.concatenate(
        [
            wq3[:, :, o0 : o0 + w].transpose(1, 0, 2).reshape(128, CB * w)
            for o0, w in _WQ_GROUPS
        ],
        axis=1,
    )
    wqp = np.ascontiguousarray(wqp)
    # SBUF images: wproj cols = [ci][o], bias cols = [ci]
    wprojp = np.ascontiguousarray(
        w_proj.T.astype(ml_dtypes.bfloat16).reshape(CB, 128, C)
        .transpose(1, 0, 2)
        .reshape(128, CB * C)
    )
    biasp = np.ascontiguousarray(
        b_proj.astype(np.float32).reshape(CB, 128).T
    )

    in_maps = []
    for core in range(N_CORES):
        b, half = divmod(core, 2)
        # own 1024 query tokens first, then the other half (key order
        # within attention is permutation-invariant)
        mine = x[b, half * NQ : (half + 1) * NQ].T
        other = x[b, (1 - half) * NQ : (2 - half) * NQ].T
        xTc = np.concatenate([mine, other], axis=1).astype(ml_dtypes.bfloat16)
        # pack to the SBUF image: cols = [tch][ci][t]
        xTp = np.ascontiguousarray(
            xTc.reshape(CB, 128, 4, 512).transpose(1, 2, 0, 3).reshape(128, CB * N)
        )
        in_maps.append({"xTp": xTp, "wqp": wqp, "wprojp": wprojp, "biasp": biasp})

    res = run_bass_kernel_spmd(nc, in_maps, core_ids=list(range(N_CORES)))

    out = np.empty((B, N, C), dtype=np.float32)
    for core in range(N_CORES):
        b, half = divmod(core, 2)
        out[b, half * NQ : (half + 1) * NQ, :] = (
            res.results[core]["outT"].astype(np.float32).T
        )
    return out



# revision 4
# speedup vs baseline: 1.1231x; 1.1231x over previous
"""Distributed multi-head attention for TRN2 (8 NeuronCores).

Reference computation (per batch b):
    qkv = x @ w_qkv.T                         # (N, 3C)
    q, k, v = split/reshape to (H, N, D)
    attn = softmax(q @ k.T * D**-0.5)         # per head
    out = (attn @ v) reassembled to (N, C)
    out = out @ w_proj.T + b_proj

Sharding: 8 cores = 4 batches x 2 HEAD-halves (tensor parallel). Each
core computes q/k/v for its own 6 heads over all 2048 tokens (no
duplicated projection work), full attention for those heads, and a
PARTIAL output projection over its 384 channels (+ bias/2). The host
sums the two partial projections of each batch - zero cross-core comm.

Layout strategy (no on-chip transposes):
  - host passes x^T and w_qkv^T slices so projections contract over
    partitions; q,k are produced d-major; scores are computed
    transposed ([keys, queries]) which is the layout attn@v consumes
  - softmax needs no max-subtraction (scores ~ N(0,1), fp32 exp range)
  - attn@v is COLUMN-TILED: the two heads of a pair run concurrently
    in the 128x128 PE array (head A -> output partitions 0:64, head B
    -> 64:128), doubling attn@v throughput vs a 65-wide ones-column
    form
  - softmax denominators come from 4-way column-tiled ones-matmuls
    (M=1 at col strips 0/32/64/96: head x kb-parity) accumulating in
    one PSUM bank across the pass
  - normalization: DVE sums the two strip partials per head, takes the
    reciprocal, a K=1 ones-matmul broadcasts 1/den across partitions,
    and one [128,512] DVE multiply normalizes both heads at once

Schedule: ACT (softmax exp, ~1.34us per 128x1024 tile) is the
steady-state bottleneck. 12 passes = 3 head pairs x 4 query quarters
(pair-major). Per pass and 2-kb step: scores alternate PE row groups;
exp(step) overlaps scores(step+1) via two PSUM score tiles; attn@v +
den lag one step. Fillers keep the PE busy under ACT: pass 0 produces
v just-in-time and pair-0 k; passes 1-3 drain remaining k/q blocks;
the last pair's passes run the partial output projection for completed
quarters. PSUM: scores 2x2 banks + acc 1 + den 1 + filler 2 = 8.

Self-contained: hardcodes B=4, N=2048, C=768, H=12, D=64.
"""

import numpy as np
import ml_dtypes

import concourse.bass as bass
import concourse.mybir as mybir
from concourse import bacc
from concourse.tile import TileContext
from concourse.bass_utils import run_bass_kernel_spmd

F32 = mybir.dt.float32
F16 = mybir.dt.float16
BF16 = mybir.dt.bfloat16
EXP = mybir.ActivationFunctionType.Exp

B, N, C = 4, 2048, 768
H, D = 12, 64
SCALE = float(D) ** -0.5  # 0.125
HC = H // 2  # 6 heads per core
CO = HC * D  # 384 own channels
HB = HC // 2  # 3 head pairs per core
CB = C // 128  # 6 input c-chunks
COB = CO // 128  # 3 own-channel chunks
TB = N // 128  # 16 token blocks
VW = CO  # 384: v block width

N_CORES = 8

# w_qkv^T column groups (own-head slice), in consumption order:
# pair-0 k, pair-0 q, all v, then k/q for pairs 1..2. Each group holds
# its column range for all six 128-row input chunks, contiguously.
# (o0 is the offset in the 3*CO-wide own-qkv column space: q [0,CO),
# k [CO,2CO), v [2CO,3CO).)
_WQ_GROUPS = [(CO, 128), (0, 128), (2 * CO, CO)]
for _ob in range(1, COB):
    _WQ_GROUPS.append((CO + _ob * 128, 128))
    _WQ_GROUPS.append((_ob * 128, 128))
_WQ_BASE = {}
_cur = 0
for _o0, _w in _WQ_GROUPS:
    _WQ_BASE[_o0] = (_cur, _w)
    _cur += CB * _w


def _build():
    nc = bacc.Bacc(None, target_bir_lowering=False)

    # host-packed SBUF images: xTp cols = [tch][ci][t]; wqp cols grouped
    # in consumption order (see _WQ_GROUPS)
    xTp = nc.declare_dram_parameter("xTp", [128, CB * N], BF16, isOutput=False)
    wqp = nc.declare_dram_parameter("wqp", [128, CB * 3 * CO], BF16, isOutput=False)
    wprojp = nc.declare_dram_parameter("wprojp", [128, COB * C], BF16, isOutput=False)
    biasp = nc.declare_dram_parameter("biasp", [128, CB], F32, isOutput=False)
    outT = nc.declare_dram_parameter("outT", [C, N], F16, isOutput=True)

    with TileContext(nc) as tc:
        with (
            tc.tile_pool(name="per", bufs=1) as per,
            tc.tile_pool(name="p23", bufs=1) as p23,
            tc.tile_pool(name="hp", bufs=6) as hp,
            tc.tile_pool(name="mi", bufs=3) as mi,
            tc.tile_pool(name="op", bufs=2) as op_pool,
            tc.tile_pool(name="ps", bufs=2, space="PSUM") as ps2,
            tc.tile_pool(name="psa", bufs=1, space="PSUM") as ps1,
        ):
            # ---- persistent tiles -------------------------------------
            qT_sb = per.tile([128, HB * N], BF16)  # q^T  [2 heads/blk, 2048]
            kT_sb = per.tile([128, HB * N], BF16)  # k^T  [2 heads/blk, 2048]
            v_sb = per.tile([128, TB * VW], BF16)  # v token-major
            bias_sb = per.tile([128, CB], F32)
            ones_sb = per.tile([128, 1], BF16)  # den stationary
            onesK1 = per.tile([1, 64], BF16)  # epi broadcast stationary
            attnT_sb = p23.tile([128, HB * N], BF16)  # attn out^T
            wproj_sb = p23.tile([128, COB * C], BF16)

            nc.vector.memset(ones_sb[:, :], 1.0)
            nc.vector.memset(onesK1[:, :], 1.0)

            # weights + activations pools, closed once the projection
            # filler has consumed them
            wqxt = (tc.tile_pool(name="wq", bufs=1), tc.tile_pool(name="xt", bufs=4))
            wq_pool = wqxt[0].__enter__()
            xt_pool = wqxt[1].__enter__()

            wqkv_sb = wq_pool.tile([128, CB * 3 * CO], BF16)
            xts = [
                xt_pool.tile([128, CB * 512], BF16, tag="xt", name=f"xt{t}")
                for t in range(4)
            ]

            def _dma_xt(tch):
                nc.sync.dma_start(
                    out=xts[tch][:, :],
                    in_=xTp[:, tch * CB * 512 : (tch + 1) * CB * 512],
                )

            def _dma_wq(gi):
                o0, w = _WQ_GROUPS[gi]
                base, _ = _WQ_BASE[o0]
                nc.sync.dma_start(
                    out=wqkv_sb[:, base : base + CB * w],
                    in_=wqp[:, base : base + CB * w],
                )

            # consumption order: chunk 0 + pair-0 k/q cols + v cols first
            # (v units run JIT in pass 0), then remaining token chunks,
            # later pairs' k/q cols
            _dma_xt(0)
            _dma_wq(0)
            _dma_wq(1)
            _dma_wq(2)
            for t in range(1, 4):
                _dma_xt(t)
            for gi in range(3, len(_WQ_GROUPS)):
                _dma_wq(gi)

            def wq(ci, o0, width):
                if o0 >= 2 * CO:
                    base, gw = _WQ_BASE[2 * CO]
                    off = o0 - 2 * CO
                else:
                    base, gw = _WQ_BASE[o0]
                    off = 0
                return wqkv_sb[:, base + ci * gw + off : base + ci * gw + off + width]

            # phase-2/3-only weights: after the critical-path DMAs
            nc.sync.dma_start(out=bias_sb[:, :], in_=biasp[:, :])
            nc.sync.dma_start(out=wproj_sb[:, :], in_=wprojp[:, :])

            # ---- projection work units (PE filler) --------------------
            def kq_unit(hb, tch, is_q):
                """one k^T (or q^T) pair-block: 128 rows, 512 tokens"""
                t0 = tch * 512
                kind = "q" if is_q else "k"
                psv = ps2.tile(
                    [128, 512], F32, tag="psV", bufs=2, name=f"{kind}{hb}_{tch}"
                )
                for ci in range(CB):
                    nc.tensor.matmul(
                        psv[:, :],
                        wq(ci, (0 if is_q else CO) + hb * 128, 128),
                        xts[tch][:, ci * 512 : (ci + 1) * 512],
                        start=(ci == 0),
                        stop=(ci == CB - 1),
                    )
                dst = qT_sb if is_q else kT_sb
                nc.vector.tensor_copy(
                    dst[:, hb * N + t0 : hb * N + t0 + 512], psv[:, :]
                )

            def v_unit(t128):
                """one v block: 128 tokens x all 384 own v-dims"""
                tch, tb = divmod(t128, 4)
                psv = ps2.tile([128, 512], F32, tag="psV", bufs=2, name=f"v{t128}")
                for ci in range(CB):
                    nc.tensor.matmul(
                        psv[:, :VW],
                        xts[tch][:, ci * 512 + tb * 128 : ci * 512 + (tb + 1) * 128],
                        wq(ci, 2 * CO, VW),
                        start=(ci == 0),
                        stop=(ci == CB - 1),
                    )
                nc.vector.tensor_copy(
                    v_sb[:, t128 * VW : (t128 + 1) * VW], psv[:, :VW]
                )

            def proj_unit(ob, qc):
                """partial out-proj: out-chunk ob, 512 queries, own 384 c"""
                psv = ps2.tile([128, 512], F32, tag="psV", bufs=2, name=f"pr{ob}_{qc}")
                for cb in range(COB):
                    nc.tensor.matmul(
                        psv[:, :],
                        wproj_sb[:, cb * C + ob * 128 : cb * C + (ob + 1) * 128],
                        attnT_sb[:, cb * N + qc * 512 : cb * N + (qc + 1) * 512],
                        start=(cb == 0),
                        stop=(cb == COB - 1),
                    )
                ot = op_pool.tile([128, 512], F16, tag="out")
                nc.vector.tensor_scalar_add(
                    ot[:, :], psv[:, :], bias_sb[:, ob : ob + 1]
                )
                nc.sync.dma_start(
                    out=outT[ob * 128 : (ob + 1) * 128, qc * 512 : (qc + 1) * 512],
                    in_=ot[:, :],
                )

            # generic filler queue for passes 1-3 (k/q blocks) and the
            # last pair's passes (partial projection)
            fill_queue = []
            fill_queue.append(("q", 0, 2))
            fill_queue.append(("k", 1, 0))
            fill_queue.append(("k", 1, 1))
            fill_queue.append(("q", 0, 3))
            fill_queue.append(("k", 1, 2))
            fill_queue.append(("k", 1, 3))
            fill_queue.append(("q", 1, 0))
            fill_queue.append(("q", 1, 1))
            fill_queue.append(("k", 2, 0))
            fill_queue.append(("k", 2, 1))
            fill_queue.append(("k", 2, 2))
            fill_queue.append(("k", 2, 3))
            fill_queue.append(("q", 1, 2))
            fill_queue.append(("q", 1, 3))
            fill_queue.append(("q", 2, 0))
            fill_queue.append(("q", 2, 1))
            fill_queue.append(("q", 2, 2))
            fill_queue.append(("q", 2, 3))
            proj_queue = []

            def fill_one():
                if fill_queue:
                    kind, hb, tch = fill_queue.pop(0)
                    kq_unit(hb, tch, kind == "q")
                elif proj_queue:
                    ob, qc = proj_queue.pop(0)
                    proj_unit(ob, qc)

            # ---- attention machinery ----------------------------------
            def epi_pe(hb_, qc_, rows_):
                """PE part of the normalization epilogue: broadcast the
                two heads' 1/den rows to 64 partitions each (col-tiled),
                then one DVE multiply normalizes the whole pair block."""
                psb = ps2.tile(
                    [128, 512], F32, tag="psV", bufs=2, name=f"psb{hb_}_{qc_}"
                )
                for hh_ in range(2):
                    nc.tensor.matmul(
                        psb[64 * hh_ : 64 * hh_ + 64, :],
                        onesK1[:, :],
                        rows_[1 + hh_][:, :],
                        start=True,
                        stop=True,
                        tile_position=(0, 64 * hh_),
                    )
                nc.vector.tensor_mul(
                    attnT_sb[:, hb_ * N + qc_ * 512 : hb_ * N + (qc_ + 1) * 512],
                    psb[:, :],
                    rows_[0][:, :],
                )

            def emit_pass(hb, qc, pend, filler):
                """One (head pair, query quarter) attention pass.
                filler(step) emits PE filler work for step in 0..7."""
                q0 = hb * N + qc * 512
                acc = ps1.tile([128, 512], F32, tag="psA", name=f"acc{hb}_{qc}")
                den = ps1.tile([128, 512], F32, tag="psD", name=f"den{hb}_{qc}")

                def av_mms(pkb, ppb):
                    for hh in range(2):
                        vs = pkb * VW + (2 * hb + hh) * D
                        nc.tensor.matmul(
                            acc[64 * hh : 64 * hh + 64, :],
                            v_sb[:, vs : vs + D],
                            ppb[:, hh * 512 : (hh + 1) * 512],
                            start=(pkb == 0),
                            stop=(pkb == TB - 1),
                            tile_position=(0, 64 * hh),
                        )

                def den_mms(prev_):
                    # 4-way col-tiled ones-matmuls: strips 0/32 <- head A
                    # (even/odd kb), strips 64/96 <- head B
                    (pk0, pb0), (_pk1, pb1) = prev_
                    first = pk0 == 0
                    last = pk0 == TB - 2
                    for hh in range(2):
                        for j, pbx in enumerate((pb0, pb1)):
                            p0 = 64 * hh + 32 * j
                            nc.tensor.matmul(
                                den[p0 : p0 + 1, :],
                                ones_sb[:, :],
                                pbx[:, hh * 512 : (hh + 1) * 512],
                                start=first,
                                stop=last,
                                tile_position=(0, p0),
                            )

                # epilogue of the previous pass: emitted first so filler
                # (projection) units that read attnT come after it
                if pend is not None:
                    epi_pe(*pend)

                prev = []
                for step in range(8):
                    kb2 = 2 * step
                    scs = []
                    for kb in (kb2, kb2 + 1):
                        sc = ps2.tile(
                            [128, 1024], F32, tag="psS", bufs=2,
                            name=f"sc{hb}_{qc}_{kb}",
                        )
                        for hh in range(2):
                            p0 = 64 * hh
                            nc.tensor.matmul(
                                sc[:, hh * 512 : (hh + 1) * 512],
                                kT_sb[
                                    p0 : p0 + 64,
                                    hb * N + kb * 128 : hb * N + (kb + 1) * 128,
                                ],
                                qT_sb[p0 : p0 + 64, q0 : q0 + 512],
                                start=True,
                                stop=True,
                                tile_position=(p0, 0),
                            )
                        scs.append(sc)
                    filler(step)
                    if prev:
                        for pkb, ppb in prev:
                            av_mms(pkb, ppb)
                        den_mms(prev)
                    prev = []
                    for i, kb in enumerate((kb2, kb2 + 1)):
                        pb = hp.tile([128, 1024], BF16, tag="probs")
                        nc.scalar.activation(
                            pb[:, :], scs[i][:, :], EXP, scale=SCALE
                        )
                        prev.append((kb, pb))
                # drain attn@v + den for the last two k-blocks
                for pkb, ppb in prev:
                    av_mms(pkb, ppb)
                den_mms(prev)
                # epilogue DVE part: free acc/den banks quickly
                cpy = mi.tile([128, 512], F32, tag="cpy")
                nc.vector.tensor_copy(cpy[:, :], acc[:, :])
                rows = [cpy]
                for hh in range(2):
                    dcp = mi.tile([1, 512], F32, tag="dcp")
                    nc.vector.tensor_copy(dcp[:, :], den[64 * hh + 32 : 64 * hh + 33, :])
                    dsum = mi.tile([1, 512], F32, tag="dsum")
                    nc.vector.tensor_add(
                        dsum[:, :],
                        den[64 * hh : 64 * hh + 1, :],
                        dcp[:, :],
                    )
                    rec = mi.tile([1, 512], F32, tag="rec")
                    nc.vector.reciprocal_approx_fast(rec[:, :], dsum[:, :])
                    row = mi.tile([1, 512], BF16, tag="row")
                    nc.vector.tensor_copy(row[:, :], rec[:, :])
                    rows.append(row)
                return (hb, qc, rows)

            # ---- pre-phase: first k/q blocks for head pair 0 ----------
            kq_unit(0, 0, False)
            kq_unit(0, 0, True)

            # pass 0 filler: v blocks JIT (2/step; v(kb) is consumed by
            # attn@v at step kb//2+1), pair-0 k chunks early enough for
            # scores (tch t needed at step 2t), q(0,1) for pass 1
            def fill_pass0(step):
                v_unit(2 * step)
                v_unit(2 * step + 1)
                if step in (0, 2, 4):
                    kq_unit(0, step // 2 + 1, False)
                if step == 6:
                    kq_unit(0, 1, True)

            # passes 1-3: drain fill_queue at 6 units per pass
            def fill_kq(step):
                if step not in (3, 7):
                    fill_one()

            # last pair's passes (9-11): 6 proj units per pass
            def fill_proj(step):
                if step not in (0, 7):
                    fill_one()

            def fill_none(step):
                pass

            pend = None
            for pi in range(12):
                hb, qc = divmod(pi, 4)
                if pi == 0:
                    filler = fill_pass0
                elif pi <= 3:
                    filler = fill_kq
                elif hb == HB - 1 and qc >= 1:
                    for ob in range(CB):
                        proj_queue.append((ob, qc - 1))
                    filler = fill_proj
                else:
                    filler = fill_none
                pend = emit_pass(hb, qc, pend, filler)
            epi_pe(*pend)

            wqxt[1].__exit__(None, None, None)
            wqxt[0].__exit__(None, None, None)

            # ---- tail: partial out-proj for the last quarter ----------
            for ob in range(CB):
                proj_unit(ob, 3)

    nc.finalize()
    return nc


_NC_CACHE = []


def _get_nc():
    if not _NC_CACHE:
        _NC_CACHE.append(_build())
    return _NC_CACHE[0]


def kernel(x, w_qkv, w_proj, b_proj):
    x = np.asarray(x, dtype=np.float32)
    w_qkv = np.asarray(w_qkv, dtype=np.float32)
    w_proj = np.asarray(w_proj, dtype=np.float32)
    b_proj = np.asarray(b_proj, dtype=np.float32)

    nc = _get_nc()

    in_maps = []
    for core in range(N_CORES):
        b, hg = divmod(core, 2)
        # x^T packed [tch][ci][t] (identical for both cores of a batch)
        xT = x[b].T.astype(ml_dtypes.bfloat16)  # [C, N]
        xTp = np.ascontiguousarray(
            xT.reshape(CB, 128, 4, 512).transpose(1, 2, 0, 3).reshape(128, CB * N)
        )
        # own-head slice of w_qkv rows, mapped into [q | k | v] x CO space
        r0 = hg * CO
        wslice = np.concatenate(
            [
                w_qkv[r0 : r0 + CO],                  # q rows
                w_qkv[C + r0 : C + r0 + CO],          # k rows
                w_qkv[2 * C + r0 : 2 * C + r0 + CO],  # v rows
            ],
            axis=0,
        )  # [3*CO, C]
        wT = wslice.T.astype(ml_dtypes.bfloat16)  # [C, 3*CO]
        w3 = np.ascontiguousarray(wT).reshape(CB, 128, 3 * CO)
        wqp = np.concatenate(
            [
                w3[:, :, o0 : o0 + w].transpose(1, 0, 2).reshape(128, CB * w)
                for o0, w in _WQ_GROUPS
            ],
            axis=1,
        )
        wqp = np.ascontiguousarray(wqp)
        # w_proj columns for own channels, rows = c-chunks
        wp = w_proj[:, r0 : r0 + CO].T.astype(ml_dtypes.bfloat16)  # [CO, C]
        wprojp = np.ascontiguousarray(
            wp.reshape(COB, 128, C).transpose(1, 0, 2).reshape(128, COB * C)
        )
        biasp = np.ascontiguousarray(
            (0.5 * b_proj).astype(np.float32).reshape(CB, 128).T
        )
        in_maps.append({"xTp": xTp, "wqp": wqp, "wprojp": wprojp, "biasp": biasp})

    res = run_bass_kernel_spmd(nc, in_maps, core_ids=list(range(N_CORES)))

    out = np.empty((B, N, C), dtype=np.float32)
    for b in range(B):
        pa = res.results[2 * b]["outT"].astype(np.float32)
        pb = res.results[2 * b + 1]["outT"].astype(np.float32)
        out[b] = (pa + pb).T
    return out


# revision 5
# speedup vs baseline: 1.2659x; 1.1272x over previous
"""Distributed multi-head attention for TRN2 (8 NeuronCores).

Reference computation (per batch b):
    qkv = x @ w_qkv.T                         # (N, 3C)
    q, k, v = split/reshape to (H, N, D)
    attn = softmax(q @ k.T * D**-0.5)         # per head
    out = (attn @ v) reassembled to (N, C)
    out = out @ w_proj.T + b_proj

Sharding: 8 cores = 4 batches x 2 HEAD-halves (tensor parallel). Each
core computes q/k/v for its own 6 heads over all 2048 tokens (no
duplicated projection work), full attention for those heads, and a
PARTIAL output projection over its 384 channels (+ bias/2). The host
sums the two partial projections of each batch - zero cross-core comm.

Layout strategy (no on-chip transposes):
  - host passes x^T and w_qkv^T slices so projections contract over
    partitions; q,k are produced d-major; scores are computed
    transposed ([keys, queries]) which is the layout attn@v consumes
  - softmax needs no max-subtraction (scores ~ N(0,1), fp32 exp range)
  - attn@v is COLUMN-TILED: the two heads of a pair run concurrently
    in the 128x128 PE array (head A -> output partitions 0:64, head B
    -> 64:128), doubling attn@v throughput vs a 65-wide ones-column
    form
  - softmax denominators come from 4-way column-tiled ones-matmuls
    (M=1 at col strips 0/32/64/96: head x kb-parity) accumulating in
    one PSUM bank across the pass
  - normalization: DVE sums the two strip partials per head, takes the
    reciprocal, a K=1 ones-matmul broadcasts 1/den across partitions,
    and one [128,512] DVE multiply normalizes both heads at once

Schedule: ACT (softmax exp, ~1.12us per 128x1024 tile) is the
steady-state bottleneck; the whole PE schedule is built to never let
it starve. 12 passes = 3 head pairs x 4 query quarters (pair-major),
SOFTWARE-PIPELINED ACROSS PASSES: each pass emits its first scores
before draining the previous pass's last attn@v/den and epilogue, so
the ACT queue never empties at pass boundaries. Per 2-kb step: scores
alternate PE row groups; attn@v + den lag one step. PE fillers are
sized to fit the per-step exp budget (k/q units split into 3-matmul
halves): pass 0 produces v just-in-time, passes 1-8 drain k/q blocks,
passes 9-11 run the partial output projection for completed quarters.
PSUM: scores 2x2 banks + acc 1 + den 1 + filler 2 = 8.

Self-contained: hardcodes B=4, N=2048, C=768, H=12, D=64.
"""

import numpy as np
import ml_dtypes

import concourse.bass as bass
import concourse.mybir as mybir
from concourse import bacc
from concourse.tile import TileContext
from concourse.bass_utils import run_bass_kernel_spmd

F32 = mybir.dt.float32
F16 = mybir.dt.float16
BF16 = mybir.dt.bfloat16
EXP = mybir.ActivationFunctionType.Exp

B, N, C = 4, 2048, 768
H, D = 12, 64
SCALE = float(D) ** -0.5  # 0.125
HC = H // 2  # 6 heads per core
CO = HC * D  # 384 own channels
HB = HC // 2  # 3 head pairs per core
CB = C // 128  # 6 input c-chunks
COB = CO // 128  # 3 own-channel chunks
TB = N // 128  # 16 token blocks
VW = CO  # 384: v block width

N_CORES = 8

# w_qkv^T column groups (own-head slice), in consumption order:
# pair-0 k, pair-0 q, all v, then k/q for pairs 1..2. Each group holds
# its column range for all six 128-row input chunks, contiguously.
# (o0 is the offset in the 3*CO-wide own-qkv column space: q [0,CO),
# k [CO,2CO), v [2CO,3CO).)
_WQ_GROUPS = [(CO, 128), (0, 128), (2 * CO, CO)]
for _ob in range(1, COB):
    _WQ_GROUPS.append((CO + _ob * 128, 128))
    _WQ_GROUPS.append((_ob * 128, 128))
_WQ_BASE = {}
_cur = 0
for _o0, _w in _WQ_GROUPS:
    _WQ_BASE[_o0] = (_cur, _w)
    _cur += CB * _w


def _build():
    nc = bacc.Bacc(None, target_bir_lowering=False)

    xTp = nc.declare_dram_parameter("xTp", [128, CB * N], BF16, isOutput=False)
    wqp = nc.declare_dram_parameter("wqp", [128, CB * 3 * CO], BF16, isOutput=False)
    wprojp = nc.declare_dram_parameter("wprojp", [128, COB * C], BF16, isOutput=False)
    biasp = nc.declare_dram_parameter("biasp", [128, CB], F32, isOutput=False)
    outT = nc.declare_dram_parameter("outT", [C, N], F16, isOutput=True)

    with TileContext(nc) as tc:
        with (
            tc.tile_pool(name="per", bufs=1) as per,
            tc.tile_pool(name="p23", bufs=1) as p23,
            tc.tile_pool(name="hp", bufs=8) as hp,
            tc.tile_pool(name="mi", bufs=3) as mi,
            tc.tile_pool(name="op", bufs=2) as op_pool,
            tc.tile_pool(name="ps", bufs=2, space="PSUM") as ps2,
            tc.tile_pool(name="psa", bufs=1, space="PSUM") as ps1,
        ):
            # ---- persistent tiles -------------------------------------
            qT_sb = per.tile([128, HB * N], BF16)  # q^T  [2 heads/blk, 2048]
            kT_sb = per.tile([128, HB * N], BF16)  # k^T  [2 heads/blk, 2048]
            v_sb = per.tile([128, TB * VW], BF16)  # v token-major
            bias_sb = per.tile([128, CB], F32)
            ones_sb = per.tile([128, 1], BF16)  # den stationary
            onesK1 = per.tile([1, 64], BF16)  # epi broadcast stationary
            attnT_sb = p23.tile([128, HB * N], BF16)  # attn out^T
            wproj_sb = p23.tile([128, COB * C], BF16)

            nc.vector.memset(ones_sb[:, :], 1.0)
            nc.vector.memset(onesK1[:, :], 1.0)

            wqxt = (tc.tile_pool(name="wq", bufs=1), tc.tile_pool(name="xt", bufs=4))
            wq_pool = wqxt[0].__enter__()
            xt_pool = wqxt[1].__enter__()

            wqkv_sb = wq_pool.tile([128, CB * 3 * CO], BF16)
            xts = [
                xt_pool.tile([128, CB * 512], BF16, tag="xt", name=f"xt{t}")
                for t in range(4)
            ]

            def _dma_xt(tch):
                nc.sync.dma_start(
                    out=xts[tch][:, :],
                    in_=xTp[:, tch * CB * 512 : (tch + 1) * CB * 512],
                )

            def _dma_wq(gi):
                o0, w = _WQ_GROUPS[gi]
                base, _ = _WQ_BASE[o0]
                nc.sync.dma_start(
                    out=wqkv_sb[:, base : base + CB * w],
                    in_=wqp[:, base : base + CB * w],
                )

            # consumption order: chunk 0 + pair-0 k/q cols + v cols first
            # (v units run JIT in pass 0), then remaining token chunks,
            # later pairs' k/q cols
            _dma_xt(0)
            _dma_wq(0)
            _dma_wq(1)
            _dma_wq(2)
            for t in range(1, 4):
                _dma_xt(t)
            for gi in range(3, len(_WQ_GROUPS)):
                _dma_wq(gi)

            def wq(ci, o0, width):
                if o0 >= 2 * CO:
                    base, gw = _WQ_BASE[2 * CO]
                    off = o0 - 2 * CO
                else:
                    base, gw = _WQ_BASE[o0]
                    off = 0
                return wqkv_sb[:, base + ci * gw + off : base + ci * gw + off + width]

            nc.sync.dma_start(out=bias_sb[:, :], in_=biasp[:, :])
            nc.sync.dma_start(out=wproj_sb[:, :], in_=wprojp[:, :])

            # ---- projection work units (PE filler) --------------------
            open_kq = {}

            def kq_half(hb, tch, is_q, half):
                """half of a k^T/q^T pair-block unit (3 of 6 matmuls)"""
                key = (hb, tch, is_q)
                if half == 0:
                    open_kq[key] = ps2.tile(
                        [128, 512], F32, tag="psV", bufs=2,
                        name=f"{'q' if is_q else 'k'}{hb}_{tch}",
                    )
                psv = open_kq[key]
                for ci in range(3 * half, 3 * half + 3):
                    nc.tensor.matmul(
                        psv[:, :],
                        wq(ci, (0 if is_q else CO) + hb * 128, 128),
                        xts[tch][:, ci * 512 : (ci + 1) * 512],
                        start=(ci == 0),
                        stop=(ci == CB - 1),
                    )
                if half == 1:
                    t0 = tch * 512
                    dst = qT_sb if is_q else kT_sb
                    nc.vector.tensor_copy(
                        dst[:, hb * N + t0 : hb * N + t0 + 512], psv[:, :]
                    )
                    del open_kq[key]

            def v_unit(t128):
                """one v block: 128 tokens x all 384 own v-dims"""
                tch, tb = divmod(t128, 4)
                psv = ps2.tile([128, 512], F32, tag="psV", bufs=2, name=f"v{t128}")
                for ci in range(CB):
                    nc.tensor.matmul(
                        psv[:, :VW],
                        xts[tch][:, ci * 512 + tb * 128 : ci * 512 + (tb + 1) * 128],
                        wq(ci, 2 * CO, VW),
                        start=(ci == 0),
                        stop=(ci == CB - 1),
                    )
                nc.vector.tensor_copy(
                    v_sb[:, t128 * VW : (t128 + 1) * VW], psv[:, :VW]
                )

            def proj_mms(ob, qc, psv, cbs):
                for cb in cbs:
                    nc.tensor.matmul(
                        psv[:, :],
                        wproj_sb[:, cb * C + ob * 128 : cb * C + (ob + 1) * 128],
                        attnT_sb[:, cb * N + qc * 512 : cb * N + (qc + 1) * 512],
                        start=(cb == 0),
                        stop=(cb == COB - 1),
                    )

            def proj_drain(ob, qc, psv):
                ot = op_pool.tile([128, 512], F16, tag="out")
                nc.vector.tensor_scalar_add(
                    ot[:, :], psv[:, :], bias_sb[:, ob : ob + 1]
                )
                nc.sync.dma_start(
                    out=outT[ob * 128 : (ob + 1) * 128, qc * 512 : (qc + 1) * 512],
                    in_=ot[:, :],
                )

            def proj_unit(ob, qc):
                psv = ps2.tile([128, 512], F32, tag="psV", bufs=2, name=f"pr{ob}_{qc}")
                proj_mms(ob, qc, psv, range(COB))
                proj_drain(ob, qc, psv)

            # ---- attention machinery ----------------------------------
            def av_mms(acc, hb, pkb, ppb):
                for hh in range(2):
                    vs = pkb * VW + (2 * hb + hh) * D
                    nc.tensor.matmul(
                        acc[64 * hh : 64 * hh + 64, :],
                        v_sb[:, vs : vs + D],
                        ppb[:, hh * 512 : (hh + 1) * 512],
                        start=(pkb == 0),
                        stop=(pkb == TB - 1),
                        tile_position=(0, 64 * hh),
                    )

            def den_mms(den, prev_):
                # 4-way col-tiled ones-matmuls: strips 0/32 <- head A
                # (even/odd kb), strips 64/96 <- head B
                (pk0, pb0), (_pk1, pb1) = prev_
                first = pk0 == 0
                last = pk0 == TB - 2
                for hh in range(2):
                    for j, pbx in enumerate((pb0, pb1)):
                        p0 = 64 * hh + 32 * j
                        nc.tensor.matmul(
                            den[p0 : p0 + 1, :],
                            ones_sb[:, :],
                            pbx[:, hh * 512 : (hh + 1) * 512],
                            start=first,
                            stop=last,
                            tile_position=(0, p0),
                        )

            def carry_drain(carry):
                """drain the previous pass's last attn@v/den and compute
                its 1/den rows on DVE; returns (hb, qc, rows)."""
                acc, den, hbp, qcp, prev = carry
                for pkb, ppb in prev:
                    av_mms(acc, hbp, pkb, ppb)
                den_mms(den, prev)
                cpy = mi.tile([128, 512], F32, tag="cpy")
                nc.vector.tensor_copy(cpy[:, :], acc[:, :])
                rows = [cpy]
                for hh in range(2):
                    dcp = mi.tile([1, 512], F32, tag="dcp")
                    nc.vector.tensor_copy(
                        dcp[:, :], den[64 * hh + 32 : 64 * hh + 33, :]
                    )
                    dsum = mi.tile([1, 512], F32, tag="dsum")
                    nc.vector.tensor_add(
                        dsum[:, :], den[64 * hh : 64 * hh + 1, :], dcp[:, :]
                    )
                    rec = mi.tile([1, 512], F32, tag="rec")
                    nc.vector.reciprocal_approx_fast(rec[:, :], dsum[:, :])
                    row = mi.tile([1, 512], BF16, tag="row")
                    nc.vector.tensor_copy(row[:, :], rec[:, :])
                    rows.append(row)
                return (hbp, qcp, rows)

            def epi_pe(hb_, qc_, rows_):
                """broadcast 1/den to 64 partitions per head (col-tiled
                K=1 matmuls), then one DVE multiply -> attnT pair block"""
                psb = ps2.tile(
                    [128, 512], F32, tag="psV", bufs=2, name=f"psb{hb_}_{qc_}"
                )
                for hh_ in range(2):
                    nc.tensor.matmul(
                        psb[64 * hh_ : 64 * hh_ + 64, :],
                        onesK1[:, :],
                        rows_[1 + hh_][:, :],
                        start=True,
                        stop=True,
                        tile_position=(0, 64 * hh_),
                    )
                nc.vector.tensor_mul(
                    attnT_sb[:, hb_ * N + qc_ * 512 : hb_ * N + (qc_ + 1) * 512],
                    psb[:, :],
                    rows_[0][:, :],
                )

            def emit_pass(hb, qc, carry, filler):
                """One (head pair, query quarter) pass, software-pipelined
                across passes: step-0 scores are emitted before draining
                the carried tail of the previous pass."""
                q0 = hb * N + qc * 512
                acc = ps1.tile([128, 512], F32, tag="psA", name=f"acc{hb}_{qc}")
                den = ps1.tile([128, 512], F32, tag="psD", name=f"den{hb}_{qc}")
                epi = None
                prev = []
                for step in range(8):
                    kb2 = 2 * step
                    scs = []
                    for kb in (kb2, kb2 + 1):
                        sc = ps2.tile(
                            [128, 1024], F32, tag="psS", bufs=2,
                            name=f"sc{hb}_{qc}_{kb}",
                        )
                        for hh in range(2):
                            p0 = 64 * hh
                            nc.tensor.matmul(
                                sc[:, hh * 512 : (hh + 1) * 512],
                                kT_sb[
                                    p0 : p0 + 64,
                                    hb * N + kb * 128 : hb * N + (kb + 1) * 128,
                                ],
                                qT_sb[p0 : p0 + 64, q0 : q0 + 512],
                                start=True,
                                stop=True,
                                tile_position=(p0, 0),
                            )
                        scs.append(sc)
                    if step == 0 and carry is not None:
                        epi = carry_drain(carry)
                    if step == 2 and epi is not None:
                        epi_pe(*epi)
                    filler(step)
                    if prev:
                        for pkb, ppb in prev:
                            av_mms(acc, hb, pkb, ppb)
                        den_mms(den, prev)
                    prev = []
                    for i, kb in enumerate((kb2, kb2 + 1)):
                        pb = hp.tile([128, 1024], BF16, tag="probs")
                        nc.scalar.activation(
                            pb[:, :], scs[i][:, :], EXP, scale=SCALE
                        )
                        prev.append((kb, pb))
                return (acc, den, hb, qc, prev)

            # ---- pre-phase: first k/q blocks for head pair 0 ----------
            for half in range(2):
                kq_half(0, 0, False, half)
            for half in range(2):
                kq_half(0, 0, True, half)

            # ---- filler schedules -------------------------------------
            # pass 0: v blocks JIT (2/step) + pair-0 k halves early
            # enough for scores (k tch t needed at step 2t) + q(0,1)
            def fill_pass0(step):
                v_unit(2 * step)
                v_unit(2 * step + 1)
                if step < 6:
                    kq_half(0, step // 2 + 1, False, step % 2)
                else:
                    kq_half(0, 1, True, step % 2)

            # passes 1-8: drain kq half-queue at steps 1,3..7 (steps 0
            # and 2 are reserved for the carried drain / epilogue)
            kq_halves = []
            for kind, hb_, tch_ in [
                ("q", 0, 2), ("q", 0, 3),
                ("k", 1, 0), ("k", 1, 1), ("k", 1, 2), ("k", 1, 3),
                ("q", 1, 0), ("q", 1, 1), ("q", 1, 2), ("q", 1, 3),
                ("k", 2, 0), ("k", 2, 1), ("k", 2, 2), ("k", 2, 3),
                ("q", 2, 0), ("q", 2, 1), ("q", 2, 2), ("q", 2, 3),
            ]:
                kq_halves.append((hb_, tch_, kind == "q", 0))
                kq_halves.append((hb_, tch_, kind == "q", 1))

            def fill_kq(step):
                if step in (0, 2):
                    return
                if kq_halves:
                    kq_half(*kq_halves.pop(0))

            # passes 9-11: partial out-projection for completed quarters
            proj_queue = []

            def mk_fill_proj(counts):
                def fill_proj(step):
                    for _ in range(counts[step]):
                        if proj_queue:
                            ob_, qc_ = proj_queue.pop(0)
                            proj_unit(ob_, qc_)
                        elif kq_halves:
                            kq_half(*kq_halves.pop(0))
                return fill_proj

            # ---- the 12 passes ----------------------------------------
            carry = None
            for pi in range(12):
                hb, qc = divmod(pi, 4)
                if pi == 0:
                    filler = fill_pass0
                elif pi < 9:
                    filler = fill_kq
                else:
                    for ob in range(CB):
                        proj_queue.append((ob, qc - 1))
                    counts = (
                        [0, 0, 0, 1, 1, 1, 1, 1]
                        if pi == 9
                        else ([0, 1, 0, 1, 1, 1, 1, 1] if pi == 10
                              else [0, 1, 0, 2, 1, 1, 1, 1])
                    )
                    filler = mk_fill_proj(counts)
                carry = emit_pass(hb, qc, carry, filler)

            # ---- tail: drain last pass + quarter-3 projection ---------
            epi = carry_drain(carry)
            # overlap part of the first proj unit with the DVE epilogue
            psv0 = ps2.tile([128, 512], F32, tag="psV", bufs=2, name="pr0_3")
            proj_mms(0, 3, psv0, range(COB - 1))
            epi_pe(*epi)
            proj_mms(0, 3, psv0, [COB - 1])
            proj_drain(0, 3, psv0)
            for ob in range(1, CB):
                proj_unit(ob, 3)

            wqxt[1].__exit__(None, None, None)
            wqxt[0].__exit__(None, None, None)

    nc.finalize()
    return nc


_NC_CACHE = []


def _get_nc():
    if not _NC_CACHE:
        _NC_CACHE.append(_build())
    return _NC_CACHE[0]


def kernel(x, w_qkv, w_proj, b_proj):
    x = np.asarray(x, dtype=np.float32)
    w_qkv = np.asarray(w_qkv, dtype=np.float32)
    w_proj = np.asarray(w_proj, dtype=np.float32)
    b_proj = np.asarray(b_proj, dtype=np.float32)

    nc = _get_nc()

    in_maps = []
    for core in range(N_CORES):
        b, hg = divmod(core, 2)
        # x^T packed [tch][ci][t] (identical for both cores of a batch)
        xT = x[b].T.astype(ml_dtypes.bfloat16)  # [C, N]
        xTp = np.ascontiguousarray(
            xT.reshape(CB, 128, 4, 512).transpose(1, 2, 0, 3).reshape(128, CB * N)
        )
        # own-head slice of w_qkv rows, mapped into [q | k | v] x CO space
        r0 = hg * CO
        wslice = np.concatenate(
            [
                w_qkv[r0 : r0 + CO],                  # q rows
                w_qkv[C + r0 : C + r0 + CO],          # k rows
                w_qkv[2 * C + r0 : 2 * C + r0 + CO],  # v rows
            ],
            axis=0,
        )  # [3*CO, C]
        wT = wslice.T.astype(ml_dtypes.bfloat16)  # [C, 3*CO]
        w3 = np.ascontiguousarray(wT).reshape(CB, 128, 3 * CO)
        wqp = np.concatenate(
            [
                w3[:, :, o0 : o0 + w].transpose(1, 0, 2).reshape(128, CB * w)
                for o0, w in _WQ_GROUPS
            ],
            axis=1,
        )
        wqp = np.ascontiguousarray(wqp)
        # w_proj columns for own channels, rows = c-chunks
        wp = w_proj[:, r0 : r0 + CO].T.astype(ml_dtypes.bfloat16)  # [CO, C]
        wprojp = np.ascontiguousarray(
            wp.reshape(COB, 128, C).transpose(1, 0, 2).reshape(128, COB * C)
        )
        biasp = np.ascontiguousarray(
            (0.5 * b_proj).astype(np.float32).reshape(CB, 128).T
        )
        in_maps.append({"xTp": xTp, "wqp": wqp, "wprojp": wprojp, "biasp": biasp})

    res = run_bass_kernel_spmd(nc, in_maps, core_ids=list(range(N_CORES)))

    out = np.empty((B, N, C), dtype=np.float32)
    for b in range(B):
        pa = res.results[2 * b]["outT"].astype(np.float32)
        pb = res.results[2 * b + 1]["outT"].astype(np.float32)
        out[b] = (pa + pb).T
    return out


# revision 15
# speedup vs baseline: 1.2820x; 1.0127x over previous
"""Distributed multi-head attention for TRN2 (8 NeuronCores).

Reference computation (per batch b):
    qkv = x @ w_qkv.T                         # (N, 3C)
    q, k, v = split/reshape to (H, N, D)
    attn = softmax(q @ k.T * D**-0.5)         # per head
    out = (attn @ v) reassembled to (N, C)
    out = out @ w_proj.T + b_proj

Sharding: 8 cores = 4 batches x 2 HEAD-halves (tensor parallel). Each
core computes q/k/v for its own 6 heads over all 2048 tokens (no
duplicated projection work), full attention for those heads, and a
PARTIAL output projection over its 384 channels (+ bias/2). The host
sums the two partial projections of each batch - zero cross-core comm.

Layout strategy (no on-chip transposes):
  - host passes x^T and w_qkv^T slices so projections contract over
    partitions; q,k are produced d-major; scores are computed
    transposed ([keys, queries]) which is the layout attn@v consumes
  - softmax needs no max-subtraction (scores ~ N(0,1), fp32 exp range)
  - attn@v is COLUMN-TILED: the two heads of a pair run concurrently
    in the 128x128 PE array (head A -> output partitions 0:64, head B
    -> 64:128), doubling attn@v throughput vs a 65-wide ones-column
    form
  - softmax denominators come from 4-way column-tiled ones-matmuls
    (M=1 at col strips 0/32/64/96: head x kb-parity) accumulating in
    one PSUM bank across the pass
  - normalization: DVE sums the two strip partials per head, takes the
    reciprocal, a K=1 ones-matmul broadcasts 1/den across partitions,
    and one [128,512] DVE multiply normalizes both heads at once

Schedule: ACT (softmax exp, ~1.12us per 128x1024 tile) is the
steady-state bottleneck; the whole PE schedule is built to never let
it starve. 12 passes = 3 head pairs x 4 query quarters (pair-major),
SOFTWARE-PIPELINED ACROSS PASSES: each pass emits its first scores
before draining the previous pass's last attn@v/den and epilogue, so
the ACT queue never empties at pass boundaries. Per 2-kb step: scores
alternate PE row groups; attn@v + den lag one step. PE fillers are
sized to fit the per-step exp budget (k/q units split into 3-matmul
halves): pass 0 produces v just-in-time, passes 1-8 drain k/q blocks,
passes 9-11 run the partial output projection for completed quarters.
PSUM: scores 2x2 banks + acc 1 + den 1 + filler 2 = 8.

Self-contained: hardcodes B=4, N=2048, C=768, H=12, D=64.
"""

import numpy as np
import ml_dtypes

import concourse.bass as bass
import concourse.mybir as mybir
from concourse import bacc
from concourse.tile import TileContext
from concourse.bass_utils import run_bass_kernel_spmd

F32 = mybir.dt.float32
F16 = mybir.dt.float16
BF16 = mybir.dt.bfloat16
EXP = mybir.ActivationFunctionType.Exp

B, N, C = 4, 2048, 768
H, D = 12, 64
SCALE = float(D) ** -0.5  # 0.125
HC = H // 2  # 6 heads per core
CO = HC * D  # 384 own channels
HB = HC // 2  # 3 head pairs per core
CB = C // 128  # 6 input c-chunks
COB = CO // 128  # 3 own-channel chunks
TB = N // 128  # 16 token blocks
VW = CO  # 384: v block width

N_CORES = 8

# w_qkv^T column groups (own-head slice), in consumption order:
# pair-0 k, pair-0 q, all v, then k/q for pairs 1..2. Each group holds
# its column range for all six 128-row input chunks, contiguously.
# (o0 is the offset in the 3*CO-wide own-qkv column space: q [0,CO),
# k [CO,2CO), v [2CO,3CO).)
_WQ_GROUPS = [(CO, 128), (0, 128), (2 * CO, CO)]
for _ob in range(1, COB):
    _WQ_GROUPS.append((CO + _ob * 128, 128))
    _WQ_GROUPS.append((_ob * 128, 128))
_WQ_BASE = {}
_cur = 0
for _o0, _w in _WQ_GROUPS:
    _WQ_BASE[_o0] = (_cur, _w)
    _cur += CB * _w


def _build():
    nc = bacc.Bacc(None, target_bir_lowering=False)

    xTp = nc.declare_dram_parameter("xTp", [128, CB * N], BF16, isOutput=False)
    wqp = nc.declare_dram_parameter("wqp", [128, CB * 3 * CO], BF16, isOutput=False)
    wprojp = nc.declare_dram_parameter("wprojp", [128, COB * C], BF16, isOutput=False)
    biasp = nc.declare_dram_parameter("biasp", [128, CB], F32, isOutput=False)
    outT = nc.declare_dram_parameter("outT", [C, N], F16, isOutput=True)

    with TileContext(nc) as tc:
        with (
            tc.tile_pool(name="per", bufs=1) as per,
            tc.tile_pool(name="p23", bufs=1) as p23,
            tc.tile_pool(name="hp", bufs=8) as hp,
            tc.tile_pool(name="mi", bufs=3) as mi,
            tc.tile_pool(name="op", bufs=2) as op_pool,
            tc.tile_pool(name="ps", bufs=2, space="PSUM") as ps2,
            tc.tile_pool(name="psa", bufs=1, space="PSUM") as ps1,
        ):
            # ---- persistent tiles -------------------------------------
            qT_sb = per.tile([128, HB * N], BF16)  # q^T  [2 heads/blk, 2048]
            kT_sb = per.tile([128, HB * N], BF16)  # k^T  [2 heads/blk, 2048]
            v_sb = per.tile([128, TB * VW], BF16)  # v token-major
            bias_sb = per.tile([128, CB], F32)
            ones_sb = per.tile([128, 1], BF16)  # den stationary
            onesK1 = per.tile([1, 64], BF16)  # epi broadcast stationary
            attnT_sb = p23.tile([128, HB * N], BF16)  # attn out^T
            wproj_sb = p23.tile([128, COB * C], BF16)

            nc.vector.memset(ones_sb[:, :], 1.0)
            nc.vector.memset(onesK1[:, :], 1.0)

            wqxt = (tc.tile_pool(name="wq", bufs=1), tc.tile_pool(name="xt", bufs=4))
            wq_pool = wqxt[0].__enter__()
            xt_pool = wqxt[1].__enter__()

            wqkv_sb = wq_pool.tile([128, CB * 3 * CO], BF16)
            xts = [
                xt_pool.tile([128, CB * 512], BF16, tag="xt", name=f"xt{t}")
                for t in range(4)
            ]

            def _dma_xt(tch):
                nc.sync.dma_start(
                    out=xts[tch][:, :],
                    in_=xTp[:, tch * CB * 512 : (tch + 1) * CB * 512],
                )

            def _dma_wq(gi):
                o0, w = _WQ_GROUPS[gi]
                base, _ = _WQ_BASE[o0]
                nc.sync.dma_start(
                    out=wqkv_sb[:, base : base + CB * w],
                    in_=wqp[:, base : base + CB * w],
                )

            def _dma_xt0_chunk(c0, c1):
                nc.sync.dma_start(
                    out=xts[0][:, c0 * 512 : c1 * 512],
                    in_=xTp[:, c0 * 512 : c1 * 512],
                )

            # consumption order: pair-0 k/q cols, then chunk 0 in pieces
            # (the pre-phase k unit consumes ci in order, so compute can
            # start while the rest of chunk 0 is still in flight), v cols
            # (JIT v units in pass 0), remaining chunks, later k/q cols
            _dma_wq(0)
            _dma_wq(1)
            _dma_xt0_chunk(0, 2)
            _dma_xt0_chunk(2, 4)
            _dma_xt0_chunk(4, 6)
            _dma_wq(2)
            for t in range(1, 4):
                _dma_xt(t)
            for gi in range(3, len(_WQ_GROUPS)):
                _dma_wq(gi)

            def wq(ci, o0, width):
                if o0 >= 2 * CO:
                    base, gw = _WQ_BASE[2 * CO]
                    off = o0 - 2 * CO
                else:
                    base, gw = _WQ_BASE[o0]
                    off = 0
                return wqkv_sb[:, base + ci * gw + off : base + ci * gw + off + width]

            nc.sync.dma_start(out=bias_sb[:, :], in_=biasp[:, :])
            nc.sync.dma_start(out=wproj_sb[:, :], in_=wprojp[:, :])

            # ---- projection work units (PE filler) --------------------
            open_kq = {}

            def kq_half(hb, tch, is_q, half):
                """half of a k^T/q^T pair-block unit (3 of 6 matmuls)"""
                key = (hb, tch, is_q)
                if half == 0:
                    open_kq[key] = ps2.tile(
                        [128, 512], F32, tag="psV", bufs=2,
                        name=f"{'q' if is_q else 'k'}{hb}_{tch}",
                    )
                psv = open_kq[key]
                for ci in range(3 * half, 3 * half + 3):
                    nc.tensor.matmul(
                        psv[:, :],
                        wq(ci, (0 if is_q else CO) + hb * 128, 128),
                        xts[tch][:, ci * 512 : (ci + 1) * 512],
                        start=(ci == 0),
                        stop=(ci == CB - 1),
                    )
                if half == 1:
                    t0 = tch * 512
                    dst = qT_sb if is_q else kT_sb
                    nc.vector.tensor_copy(
                        dst[:, hb * N + t0 : hb * N + t0 + 512], psv[:, :]
                    )
                    del open_kq[key]

            def v_unit(t128):
                """one v block: 128 tokens x all 384 own v-dims"""
                tch, tb = divmod(t128, 4)
                psv = ps2.tile([128, 512], F32, tag="psV", bufs=2, name=f"v{t128}")
                for ci in range(CB):
                    nc.tensor.matmul(
                        psv[:, :VW],
                        xts[tch][:, ci * 512 + tb * 128 : ci * 512 + (tb + 1) * 128],
                        wq(ci, 2 * CO, VW),
                        start=(ci == 0),
                        stop=(ci == CB - 1),
                    )
                nc.vector.tensor_copy(
                    v_sb[:, t128 * VW : (t128 + 1) * VW], psv[:, :VW]
                )

            def proj_mms(ob, qc, psv, cbs):
                for cb in cbs:
                    nc.tensor.matmul(
                        psv[:, :],
                        wproj_sb[:, cb * C + ob * 128 : cb * C + (ob + 1) * 128],
                        attnT_sb[:, cb * N + qc * 512 : cb * N + (qc + 1) * 512],
                        start=(cb == 0),
                        stop=(cb == COB - 1),
                    )

            def proj_drain(ob, qc, psv):
                ot = op_pool.tile([128, 512], F16, tag="out")
                nc.vector.tensor_scalar_add(
                    ot[:, :], psv[:, :], bias_sb[:, ob : ob + 1]
                )
                nc.sync.dma_start(
                    out=outT[ob * 128 : (ob + 1) * 128, qc * 512 : (qc + 1) * 512],
                    in_=ot[:, :],
                )

            def proj_unit(ob, qc):
                psv = ps2.tile([128, 512], F32, tag="psV", bufs=2, name=f"pr{ob}_{qc}")
                proj_mms(ob, qc, psv, range(COB))
                proj_drain(ob, qc, psv)

            # ---- attention machinery ----------------------------------
            def av_mms(acc, hb, pkb, ppb):
                for hh in range(2):
                    vs = pkb * VW + (2 * hb + hh) * D
                    nc.tensor.matmul(
                        acc[64 * hh : 64 * hh + 64, :],
                        v_sb[:, vs : vs + D],
                        ppb[:, hh * 512 : (hh + 1) * 512],
                        start=(pkb == 0),
                        stop=(pkb == TB - 1),
                        tile_position=(0, 64 * hh),
                    )

            def den_mms(den, prev_):
                # 4-way col-tiled ones-matmuls: strips 0/32 <- head A
                # (even/odd kb), strips 64/96 <- head B
                (pk0, pb0), (_pk1, pb1) = prev_
                first = pk0 == 0
                last = pk0 == TB - 2
                for hh in range(2):
                    for j, pbx in enumerate((pb0, pb1)):
                        p0 = 64 * hh + 32 * j
                        nc.tensor.matmul(
                            den[p0 : p0 + 1, :],
                            ones_sb[:, :],
                            pbx[:, hh * 512 : (hh + 1) * 512],
                            start=first,
                            stop=last,
                            tile_position=(0, p0),
                        )

            def carry_drain(carry):
                """drain the previous pass's last attn@v/den and compute
                its 1/den rows on DVE; returns (hb, qc, (cpy, row)).

                The den strips are combined with a single partition-
                offset add: dsb[p] + dsb[32+p] puts head A's denominator
                at partition 0 and head B's at partition 64 of one tile,
                so one reciprocal + one cast serve both heads. (den's
                unwritten partitions hold 1.0 from the one-time memset,
                keeping the unused lanes finite.) The den PSUM tile is
                read by exactly one DVE op, so the next pass's den
                matmuls are released after ~0.7us."""
                acc, hbp, qcp, prev = carry
                for pkb, ppb in prev:
                    av_mms(acc, hbp, pkb, ppb)
                den_mms(den_t, prev)
                dsb = mi.tile([97, 512], F32, tag="dsb")
                nc.vector.tensor_copy(dsb[:, :], den_t[0:97, :])
                cpy = mi.tile([128, 512], F32, tag="cpy")
                nc.vector.tensor_copy(cpy[:, :], acc[:, :])
                rows = [cpy]
                for hh in range(2):
                    # mixed PSUM+SBUF operands (different base partitions
                    # are only legal when not both inputs are in SBUF)
                    dsum = mi.tile([1, 512], F32, tag="dsum")
                    nc.vector.tensor_add(
                        dsum[:, :],
                        den_t[64 * hh : 64 * hh + 1, :],
                        dsb[64 * hh + 32 : 64 * hh + 33, :],
                    )
                    rec = mi.tile([1, 512], F32, tag="rec")
                    nc.vector.reciprocal_approx_fast(rec[:, :], dsum[:, :])
                    row = mi.tile([1, 512], BF16, tag="row")
                    nc.vector.tensor_copy(row[:, :], rec[:, :])
                    rows.append(row)
                return (hbp, qcp, rows)

            def epi_pe(hb_, qc_, rows_, psb=None):
                """broadcast 1/den to 64 partitions per head (col-tiled
                K=1 matmuls), then one DVE multiply -> attnT pair block"""
                if psb is None:
                    psb = ps2.tile(
                        [128, 512], F32, tag="psV", bufs=2,
                        name=f"psb{hb_}_{qc_}",
                    )
                for hh_ in range(2):
                    nc.tensor.matmul(
                        psb[64 * hh_ : 64 * hh_ + 64, 0:512],
                        onesK1[:, :],
                        rows_[1 + hh_][:, :],
                        start=True,
                        stop=True,
                        tile_position=(0, 64 * hh_),
                    )
                nc.vector.tensor_mul(
                    attnT_sb[:, hb_ * N + qc_ * 512 : hb_ * N + (qc_ + 1) * 512],
                    psb[0:128, 0:512],
                    rows_[0][:, :],
                )

            def emit_pass(hb, qc, carry, filler):
                """One (head pair, query quarter) pass, software-pipelined
                across passes: step-0 scores are emitted before draining
                the carried tail of the previous pass."""
                q0 = hb * N + qc * 512
                acc = ps1.tile([128, 512], F32, tag="psA", name=f"acc{hb}_{qc}")
                epi = None
                prev = []
                for step in range(8):
                    kb2 = 2 * step
                    scs = []
                    for kb in (kb2, kb2 + 1):
                        sc = ps2.tile(
                            [128, 1024], F32, tag="psS", bufs=2,
                            name=f"sc{hb}_{qc}_{kb}",
                        )
                        for hh in range(2):
                            p0 = 64 * hh
                            nc.tensor.matmul(
                                sc[:, hh * 512 : (hh + 1) * 512],
                                kT_sb[
                                    p0 : p0 + 64,
                                    hb * N + kb * 128 : hb * N + (kb + 1) * 128,
                                ],
                                qT_sb[p0 : p0 + 64, q0 : q0 + 512],
                                start=True,
                                stop=True,
                                tile_position=(p0, 0),
                            )
                        scs.append(sc)
                    if step == 0 and carry is not None:
                        epi = carry_drain(carry)
                    if step == 2 and epi is not None:
                        epi_pe(*epi)
                    filler(step)
                    if prev:
                        for pkb, ppb in prev:
                            av_mms(acc, hb, pkb, ppb)
                        den_mms(den_t, prev)
                    prev = []
                    for i, kb in enumerate((kb2, kb2 + 1)):
                        pb = hp.tile([128, 1024], BF16, tag="probs")
                        nc.scalar.activation(
                            pb[:, :], scs[i][:, :], EXP, scale=SCALE
                        )
                        prev.append((kb, pb))
                return (acc, hb, qc, prev)

            # ---- pre-phase ---------------------------------------------
            # persistent den accumulator; unwritten partitions hold 1.0
            # forever so the epilogue's whole-tile ops stay finite
            den_t = ps1.tile([128, 512], F32, tag="psD", name="den")
            nc.vector.memset(den_t[:, :], 1.0)
            # first k/q blocks for head pair 0
            for half in range(2):
                kq_half(0, 0, False, half)
            for half in range(2):
                kq_half(0, 0, True, half)

            # ---- filler schedules -------------------------------------
            # pass 0: v blocks JIT (2/step) + pair-0 k halves early
            # enough for scores (k tch t needed at step 2t) + q(0,1)
            def fill_pass0(step):
                v_unit(2 * step)
                v_unit(2 * step + 1)
                if step < 6:
                    kq_half(0, step // 2 + 1, False, step % 2)
                else:
                    kq_half(0, 1, True, step % 2)

            # passes 1-8: drain kq half-queue at steps 1,3..7 (steps 0
            # and 2 are reserved for the carried drain / epilogue)
            kq_halves = []
            for kind, hb_, tch_ in [
                ("q", 0, 2), ("q", 0, 3),
                ("k", 1, 0), ("k", 1, 1), ("k", 1, 2), ("k", 1, 3),
                ("q", 1, 0), ("q", 1, 1), ("q", 1, 2), ("q", 1, 3),
                ("k", 2, 0), ("k", 2, 1), ("k", 2, 2), ("k", 2, 3),
                ("q", 2, 0), ("q", 2, 1), ("q", 2, 2), ("q", 2, 3),
            ]:
                kq_halves.append((hb_, tch_, kind == "q", 0))
                kq_halves.append((hb_, tch_, kind == "q", 1))

            def fill_kq(step):
                if step in (0, 2):
                    return
                if kq_halves:
                    kq_half(*kq_halves.pop(0))

            # passes 9-11: partial out-projection for completed quarters
            proj_queue = []

            def mk_fill_proj(counts):
                def fill_proj(step):
                    for _ in range(counts[step]):
                        if proj_queue:
                            ob_, qc_ = proj_queue.pop(0)
                            proj_unit(ob_, qc_)
                        elif kq_halves:
                            kq_half(*kq_halves.pop(0))
                return fill_proj

            # ---- the 12 passes ----------------------------------------
            carry = None
            for pi in range(12):
                hb, qc = divmod(pi, 4)
                if pi == 0:
                    filler = fill_pass0
                elif pi < 9:
                    filler = fill_kq
                else:
                    for ob in range(CB):
                        proj_queue.append((ob, qc - 1))
                    counts = (
                        [0, 0, 0, 1, 1, 1, 1, 1]
                        if pi == 9
                        else ([0, 1, 0, 1, 1, 1, 1, 1] if pi == 10
                              else [0, 1, 0, 2, 1, 1, 1, 1])
                    )
                    filler = mk_fill_proj(counts)
                carry = emit_pass(hb, qc, carry, filler)

            # ---- tail: drain last pass + quarter-3 projection ---------
            epi = carry_drain(carry)
            # overlap the first two proj units' pair-0/1 matmuls with the
            # DVE epilogue (keeps the PE warm); the epilogue broadcast
            # uses a scores-pool PSUM tile so both filler buffers stay
            # available for the projection chains
            psv0 = ps2.tile([128, 512], F32, tag="psV", bufs=2, name="pr0_3")
            proj_mms(0, 3, psv0, range(COB - 1))
            psv1 = ps2.tile([128, 512], F32, tag="psV", bufs=2, name="pr1_3")
            proj_mms(1, 3, psv1, range(COB - 1))
            psbt = ps2.tile([128, 1024], F32, tag="psS", bufs=2, name="psbt")
            epi_pe(epi[0], epi[1], epi[2], psb=psbt)
            proj_mms(0, 3, psv0, [COB - 1])
            proj_drain(0, 3, psv0)
            proj_mms(1, 3, psv1, [COB - 1])
            proj_drain(1, 3, psv1)
            for ob in range(2, CB):
                proj_unit(ob, 3)

            wqxt[1].__exit__(None, None, None)
            wqxt[0].__exit__(None, None, None)

    nc.finalize()
    return nc


_NC_CACHE = []


def _get_nc():
    if not _NC_CACHE:
        _NC_CACHE.append(_build())
    return _NC_CACHE[0]


def kernel(x, w_qkv, w_proj, b_proj):
    x = np.asarray(x, dtype=np.float32)
    w_qkv = np.asarray(w_qkv, dtype=np.float32)
    w_proj = np.asarray(w_proj, dtype=np.float32)
    b_proj = np.asarray(b_proj, dtype=np.float32)

    nc = _get_nc()

    in_maps = []
    for core in range(N_CORES):
        b, hg = divmod(core, 2)
        # x^T packed [tch][ci][t] (identical for both cores of a batch)
        xT = x[b].T.astype(ml_dtypes.bfloat16)  # [C, N]
        xTp = np.ascontiguousarray(
            xT.reshape(CB, 128, 4, 512).transpose(1, 2, 0, 3).reshape(128, CB * N)
        )
        # own-head slice of w_qkv rows, mapped into [q | k | v] x CO space
        r0 = hg * CO
        wslice = np.concatenate(
            [
                w_qkv[r0 : r0 + CO],                  # q rows
                w_qkv[C + r0 : C + r0 + CO],          # k rows
                w_qkv[2 * C + r0 : 2 * C + r0 + CO],  # v rows
            ],
            axis=0,
        )  # [3*CO, C]
        wT = wslice.T.astype(ml_dtypes.bfloat16)  # [C, 3*CO]
        w3 = np.ascontiguousarray(wT).reshape(CB, 128, 3 * CO)
        wqp = np.concatenate(
            [
                w3[:, :, o0 : o0 + w].transpose(1, 0, 2).reshape(128, CB * w)
                for o0, w in _WQ_GROUPS
            ],
            axis=1,
        )
        wqp = np.ascontiguousarray(wqp)
        # w_proj columns for own channels, rows = c-chunks
        wp = w_proj[:, r0 : r0 + CO].T.astype(ml_dtypes.bfloat16)  # [CO, C]
        wprojp = np.ascontiguousarray(
            wp.reshape(COB, 128, C).transpose(1, 0, 2).reshape(128, COB * C)
        )
        biasp = np.ascontiguousarray(
            (0.5 * b_proj).astype(np.float32).reshape(CB, 128).T
        )
        in_maps.append({"xTp": xTp, "wqp": wqp, "wprojp": wprojp, "biasp": biasp})

    res = run_bass_kernel_spmd(nc, in_maps, core_ids=list(range(N_CORES)))

    out = np.empty((B, N, C), dtype=np.float32)
    for b in range(B):
        pa = res.results[2 * b]["outT"].astype(np.float32)
        pb = res.results[2 * b + 1]["outT"].astype(np.float32)
        out[b] = (pa + pb).T
    return out
